# revision 1
# baseline (speedup 1.0000x reference)
"""GroupedQueryAttention Trainium2 Bass kernel.

Sharding: 8 cores = 2 (batch) x 4 (KV groups). Each core computes, for its
(b, g): q/k/v projections for the group's 4 query heads + 1 kv head, causal
attention, and the partial output projection ctx_g @ Wo[g-rows]. Host sums
the 4 group partials per batch and adds the bias.

All matmuls run in float32r (TF32-class) with fp32 PSUM accumulation.
Attention is computed in the S^T orientation (keys on partitions, queries on
the free dim) so exp(S^T) feeds both the PV matmul (lhsT = v, natural
layout) and the softmax row-sum (ones-matmul) with zero transposes.
"""
import sys
sys.path.insert(0, '/opt/trn_rl_repo')

import numpy as np
import concourse.bass as bass
import concourse.bacc as bacc
import concourse.tile as tile
import concourse.mybir as mybir
from concourse import bass_utils
from concourse.bass_interp import get_hw_module
from contextlib import ExitStack, nullcontext

F32 = mybir.dt.float32
F32R = mybir.dt.float32r
AF = mybir.ActivationFunctionType
ALU = mybir.AluOpType

SEQ = 2048
D = 2048
HD = 128          # head dim
NHL = 4           # query heads per core (group size)
QC = 512          # query chunk
NQC = SEQ // QC   # 4
NKT = SEQ // 128  # 16 key tiles
NDT = D // 128    # 16 contraction tiles
SCALE = 1.0 / float(np.sqrt(HD))
NEG = -1e30


def build_program(niter=1, stages=(1, 2, 3), no_ones=False):
    nc = bacc.Bacc("TRN2", target_bir_lowering=False, debug=False,
                   enable_asserts=False, num_devices=8)
    xT = nc.dram_tensor("xT", [D, SEQ], F32R, kind="ExternalInput").ap()
    Wq = nc.dram_tensor("Wq", [D, NHL * HD], F32R, kind="ExternalInput").ap()
    Wk = nc.dram_tensor("Wk", [D, HD], F32R, kind="ExternalInput").ap()
    Wv = nc.dram_tensor("Wv", [D, HD], F32R, kind="ExternalInput").ap()
    Wo = nc.dram_tensor("Wo", [NHL * HD, D], F32R, kind="ExternalInput").ap()
    NM = nc.dram_tensor("NM", [128, 128], F32, kind="ExternalInput").ap()
    ONESC = nc.dram_tensor("ONESC", [128, 1], F32R, kind="ExternalInput").ap()
    ONESR = nc.dram_tensor("ONESR", [1, 128], F32R, kind="ExternalInput").ap()
    IDENT = nc.dram_tensor("IDENT", [128, 128], F32, kind="ExternalInput").ap()
    OUT = nc.dram_tensor("out", [SEQ, D], F32, kind="ExternalOutput").ap()

    with tile.TileContext(nc) as tc:
        with (tc.For_i(0, niter, 1) if niter > 1 else nullcontext()):
          with ExitStack() as octx:
              const = octx.enter_context(tc.tile_pool(name="const", bufs=1))
              wopool = octx.enter_context(tc.tile_pool(name="wopool", bufs=1))
              resid = octx.enter_context(tc.tile_pool(name="resid", bufs=1))

              negmask = const.tile([128, 128], F32)
              onesc = const.tile([128, 1], F32R)
              onesr = const.tile([1, 128], F32R)
              ident = const.tile([128, 128], F32)
              nc.sync.dma_start(negmask[:], NM[:, :])
              nc.sync.dma_start(onesc[:], ONESC[:, :])
              nc.sync.dma_start(onesr[:], ONESR[:, :])
              nc.sync.dma_start(ident[:], IDENT[:, :])

              # Resident SBUF arrays spanning stages.
              qT = [resid.tile([128, SEQ], F32R, name=f"qT{s}", tag=f"qT{s}")
                    for s in range(NHL)]
              kT = resid.tile([128, SEQ], F32R, name="kT", tag="kT")
              # vcat[:, ki*128:+128] = v[ki*128:(ki+1)*128, :]  ([seq-in-tile, hd])
              vcat = resid.tile([128, SEQ], F32R, name="vcat", tag="vcat")
              ctx = [resid.tile([128, SEQ], F32R, name=f"ctx{s}", tag=f"ctx{s}")
                     for s in range(NHL)]

              if 1 not in stages:
                  # timing-only variants: fill resident tiles with real data
                  for s in range(NHL):
                      nc.sync.dma_start(qT[s][:], xT[0:128, :])
                      nc.sync.dma_start(ctx[s][:], xT[128:256, :])
                  nc.sync.dma_start(kT[:], xT[256:384, :])
                  nc.sync.dma_start(vcat[:], xT[384:512, :])

              # ---------------- Stage 1: projections ----------------
              with ExitStack() as s1:
                if 1 in stages:
                    wpool = s1.enter_context(tc.tile_pool(name="wpool", bufs=1))
                    xpool = s1.enter_context(tc.tile_pool(name="xpool", bufs=20))
                    vtpool = s1.enter_context(tc.tile_pool(name="vtpool", bufs=2))
                    pps = s1.enter_context(
                        tc.tile_pool(name="proj_ps", bufs=6, space="PSUM"))
                    trps = s1.enter_context(
                        tc.tile_pool(name="tr_ps", bufs=2, space="PSUM"))

                    wq_t = [wpool.tile([128, NHL * HD], F32R, name=f"wq{d}",
                                       tag=f"wq{d}") for d in range(NDT)]
                    wk_t = [wpool.tile([128, HD], F32R, name=f"wk{d}",
                                       tag=f"wk{d}") for d in range(NDT)]
                    wv_t = [wpool.tile([128, HD], F32R, name=f"wv{d}",
                                       tag=f"wv{d}") for d in range(NDT)]
                    for d in range(NDT):
                        nc.sync.dma_start(wq_t[d][:], Wq[d*128:(d+1)*128, :])
                        nc.sync.dma_start(wk_t[d][:], Wk[d*128:(d+1)*128, :])
                        nc.sync.dma_start(wv_t[d][:], Wv[d*128:(d+1)*128, :])

                    for c in range(NQC):
                        cs = slice(c * QC, (c + 1) * QC)
                        psq = [pps.tile([128, QC], F32, name=f"psq{s}_{c}",
                                        tag="proj") for s in range(NHL)]
                        psk = pps.tile([128, QC], F32, name=f"psk{c}", tag="proj")
                        psv = pps.tile([128, QC], F32, name=f"psv{c}", tag="proj")
                        for d in range(NDT):
                            xt = xpool.tile([128, QC], F32R, name=f"xt{c}_{d}",
                                            tag="xt")
                            nc.sync.dma_start(xt[:], xT[d*128:(d+1)*128, cs])
                            st = (d == 0)
                            sp = (d == NDT - 1)
                            for s in range(NHL):
                                nc.tensor.matmul(psq[s][:],
                                                 wq_t[d][:, s*HD:(s+1)*HD],
                                                 xt[:], start=st, stop=sp)
                            nc.tensor.matmul(psk[:], wk_t[d][:], xt[:],
                                             start=st, stop=sp)
                            nc.tensor.matmul(psv[:], wv_t[d][:], xt[:],
                                             start=st, stop=sp)
                        for s in range(NHL):
                            nc.any.tensor_copy(qT[s][:, cs], psq[s][:])
                        nc.any.tensor_copy(kT[:, cs], psk[:])
                        # v: evict vT chunk, then PE-transpose 128x128 blocks
                        vt = vtpool.tile([128, QC], F32, name=f"vt{c}", tag="vt")
                        nc.any.tensor_copy(vt[:], psv[:])
                        for t in range(QC // 128):
                            trp = trps.tile([128, 128], F32, name=f"tr{c}_{t}",
                                            tag="tr")
                            nc.tensor.transpose(trp[:], vt[:, t*128:(t+1)*128],
                                                ident[:])
                            col = c * QC + t * 128
                            nc.any.tensor_copy(vcat[:, col:col+128], trp[:])

              # ---------------- Stage 2: attention ----------------
              with ExitStack() as s2:
                if 2 in stages or 3 in stages:
                    epool = s2.enter_context(tc.tile_pool(name="epool", bufs=4))
                    rpool = s2.enter_context(tc.tile_pool(name="rpool", bufs=2))
                    stps = s2.enter_context(
                        tc.tile_pool(name="st_ps", bufs=3, space="PSUM"))
                    ctxps = s2.enter_context(
                        tc.tile_pool(name="ctx_ps", bufs=2, space="PSUM"))
                    rsps = s2.enter_context(
                        tc.tile_pool(name="rs_ps", bufs=2, space="PSUM"))
                    bcps = s2.enter_context(
                        tc.tile_pool(name="bc_ps", bufs=1, space="PSUM"))

                    wo_t = [wopool.tile([128, D], F32R, name=f"wo{s}",
                                        tag=f"wo{s}") for s in range(NHL)]
                    if 2 not in stages:
                        for s in range(NHL):
                            nc.sync.dma_start(wo_t[s][:],
                                              Wo[s*128:(s+1)*128, :])

                    for h in (range(NHL) if 2 in stages else ()):
                        if h == 1:
                            # prefetch Wo while attention runs
                            for s in range(NHL):
                                nc.sync.dma_start(wo_t[s][:],
                                                  Wo[s*128:(s+1)*128, :])
                        for c in range(NQC):
                            cs = slice(c * QC, (c + 1) * QC)
                            ktmax = 4 * (c + 1)
                            ctxp = ctxps.tile([128, QC], F32,
                                              name=f"ctxp{h}_{c}", tag="ctxp")
                            rsp = rsps.tile([1, QC], F32, name=f"rsp{h}_{c}",
                                            tag="rsp")
                            for ki in range(ktmax):
                                j = ki - 4 * c
                                n0 = 0 if j < 0 else 128 * j
                                ns = slice(n0, QC)
                                stt = stps.tile([128, QC], F32,
                                                name=f"st{h}_{c}_{ki}", tag="st")
                                nc.tensor.matmul(
                                    stt[:, ns], kT[:, ki*128:(ki+1)*128],
                                    qT[h][:, c*QC+n0:(c+1)*QC],
                                    start=True, stop=True)
                                if j >= 0:
                                    nc.vector.tensor_tensor(
                                        stt[:, n0:n0+128], stt[:, n0:n0+128],
                                        negmask[:], ALU.add)
                                est = epool.tile([128, QC], F32R,
                                                 name=f"est{h}_{c}_{ki}",
                                                 tag="est")
                                nc.scalar.activation(est[:, ns], stt[:, ns],
                                                     AF.Exp, scale=SCALE)
                                nc.tensor.matmul(ctxp[:, ns],
                                                 vcat[:, ki*128:(ki+1)*128],
                                                 est[:, ns],
                                                 start=(ki == 0),
                                                 stop=(ki == ktmax - 1))
                                if not no_ones:
                                    nc.tensor.matmul(rsp[:, ns], onesc[:],
                                                     est[:, ns],
                                                     start=(ki == 0),
                                                     stop=(ki == ktmax - 1))
                            if no_ones:
                                nc.vector.tensor_copy(ctx[h][:, cs], ctxp[:])
                            else:
                                recip = rpool.tile([1, QC], F32R,
                                                   name=f"recip{h}_{c}",
                                                   tag="recip")
                                with nc.allow_low_precision(
                                        reason="fp32r recip, fp32r matmul"):
                                    nc.vector.reciprocal(recip[:], rsp[:])
                                bcp = bcps.tile([128, QC], F32,
                                                name=f"bc{h}_{c}", tag="bc")
                                nc.tensor.matmul(bcp[:], onesr[:], recip[:],
                                                 start=True, stop=True)
                                nc.vector.tensor_copy(ctx[h][:, cs],
                                                      ctxp[:])
                                nc.vector.tensor_tensor(ctx[h][:, cs],
                                                        ctx[h][:, cs],
                                                        bcp[:], ALU.mult)

              # ---------------- Stage 3: output projection ----------------
              with ExitStack() as s3:
                if 3 in stages:
                    opool = s3.enter_context(tc.tile_pool(name="opool", bufs=3))
                    ops = s3.enter_context(
                        tc.tile_pool(name="out_ps", bufs=6, space="PSUM"))
                    for m in range(SEQ // 128):
                        ms = slice(m * 128, (m + 1) * 128)
                        pso = [ops.tile([128, 512], F32, name=f"pso{m}_{n}",
                                        tag="pso") for n in range(4)]
                        for s in range(NHL):
                            for n in range(4):
                                nc.tensor.matmul(pso[n][:], ctx[s][:, ms],
                                                 wo_t[s][:, n*512:(n+1)*512],
                                                 start=(s == 0),
                                                 stop=(s == NHL - 1))
                        ot = opool.tile([128, D], F32, name=f"ot{m}", tag="ot")
                        for n in range(4):
                            nc.any.tensor_copy(ot[:, n*512:(n+1)*512], pso[n][:])
                        nc.sync.dma_start(OUT[ms, :], ot[:])

    nc.compile()
    nc.m = get_hw_module(nc.m)
    return nc


_NC = None


def _get_nc():
    global _NC
    if _NC is None:
        _NC = build_program()
    return _NC


def _consts():
    negmask = np.where(np.arange(128)[:, None] <= np.arange(128)[None, :],
                       0.0, NEG).astype(np.float32)
    return {
        "NM": negmask,
        "ONESC": np.ones((128, 1), np.float32),
        "ONESR": np.ones((1, 128), np.float32),
        "IDENT": np.eye(128, dtype=np.float32),
    }


def kernel(x, Wq, Wk, Wv, Wo, bo):
    x = np.asarray(x, np.float32)
    Wq = np.asarray(Wq, np.float32)
    Wk = np.asarray(Wk, np.float32)
    Wv = np.asarray(Wv, np.float32)
    Wo = np.asarray(Wo, np.float32)
    bo = np.asarray(bo, np.float32)
    b = x.shape[0]
    nc = _get_nc()
    consts = _consts()
    xTs = [np.ascontiguousarray(x[i].T) for i in range(b)]
    in_maps = []
    for i in range(8):
        bi, g = i // 4, i % 4
        in_maps.append({
            "xT": xTs[bi],
            "Wq": np.ascontiguousarray(Wq[:, g*512:(g+1)*512]),
            "Wk": np.ascontiguousarray(Wk[:, g*128:(g+1)*128]),
            "Wv": np.ascontiguousarray(Wv[:, g*128:(g+1)*128]),
            "Wo": np.ascontiguousarray(Wo[g*512:(g+1)*512, :]),
            **consts,
        })
    res = bass_utils.run_bass_kernel_spmd(nc, in_maps,
                                          core_ids=list(range(8)),
                                          trace=False)
    out = np.zeros((b, SEQ, D), np.float32)
    for i in range(8):
        bi = i // 4
        out[bi] += res.results[i]["out"]
    out += bo[None, None, :]
    return out



# revision 16
# speedup vs baseline: 5.2678x; 5.2678x over previous
"""GroupedQueryAttention Trainium2 Bass kernel (v2, chunk-pipelined).

Sharding: 8 cores = 2 (batch) x 4 (KV groups). Each core computes, for its
(b, g): q/k/v projections for the group's 4 query heads + 1 kv head, causal
attention, and the partial output projection ctx_g @ Wo[g-rows]. Host sums
the 4 group partials per batch and adds the bias.

v2 layout: one chunk-major loop (512 queries per chunk) that runs
projection -> attention -> output projection per chunk, so the Tile
scheduler can overlap the ACT-bound attention phase of chunk c with the
PE-bound projection of chunk c+1 / out-projection of chunk c-1.
Softmax normalization is off the critical path: ctx is evicted
unnormalized, denominators go through reciprocal_approx_fast (DVE) and a
deferred broadcast-matmul + multiply.
"""
import sys
sys.path.insert(0, '/opt/trn_rl_repo')

import numpy as np
import concourse.bass as bass
import concourse.bacc as bacc
import concourse.tile as tile
import concourse.mybir as mybir
from concourse import bass_utils
from concourse.bass_interp import get_hw_module
from contextlib import ExitStack, nullcontext

F32 = mybir.dt.float32
F32R = mybir.dt.float32r
AF = mybir.ActivationFunctionType
ALU = mybir.AluOpType

SEQ = 2048
D = 2048
HD = 128          # head dim
NHL = 4           # query heads per core (group size)
QC = 512          # query chunk
NQC = SEQ // QC   # 4
NDT = D // 128    # 16 contraction tiles
SCALE = 1.0 / float(np.sqrt(HD))
NEG = -1e30


def build_program(niter=1):
    nc = bacc.Bacc("TRN2", target_bir_lowering=False, debug=False,
                   enable_asserts=False, num_devices=8)
    xT = nc.dram_tensor("xT", [D, SEQ], F32R, kind="ExternalInput").ap()
    Wq = nc.dram_tensor("Wq", [D, NHL * HD], F32R, kind="ExternalInput").ap()
    Wk = nc.dram_tensor("Wk", [D, HD], F32R, kind="ExternalInput").ap()
    Wv = nc.dram_tensor("Wv", [D, HD], F32R, kind="ExternalInput").ap()
    Wo = nc.dram_tensor("Wo", [NHL * HD, D], F32R, kind="ExternalInput").ap()
    NM = nc.dram_tensor("NM", [128, 128], F32, kind="ExternalInput").ap()
    ONESC = nc.dram_tensor("ONESC", [128, 1], F32R, kind="ExternalInput").ap()
    ONESR = nc.dram_tensor("ONESR", [1, 128], F32R, kind="ExternalInput").ap()
    IDENT = nc.dram_tensor("IDENT", [128, 128], F32, kind="ExternalInput").ap()
    OUT = nc.dram_tensor("out", [SEQ, D], F32, kind="ExternalOutput").ap()

    with tile.TileContext(nc) as tc:
        with (tc.For_i(0, niter, 1) if niter > 1 else nullcontext()):
          with ExitStack() as octx:
            const = octx.enter_context(tc.tile_pool(name="const", bufs=1))
            wpool = octx.enter_context(tc.tile_pool(name="wpool", bufs=1))
            resid = octx.enter_context(tc.tile_pool(name="resid", bufs=1))
            xpool = octx.enter_context(tc.tile_pool(name="xpool", bufs=18))
            bcpool = octx.enter_context(tc.tile_pool(name="bcpool", bufs=2))
            qpool = octx.enter_context(tc.tile_pool(name="qpool", bufs=8))
            cxpool = octx.enter_context(tc.tile_pool(name="cxpool", bufs=8))
            epool = octx.enter_context(tc.tile_pool(name="epool", bufs=3))
            vtpool = octx.enter_context(tc.tile_pool(name="vtpool", bufs=2))
            rpool = octx.enter_context(tc.tile_pool(name="rpool", bufs=8))
            opool = octx.enter_context(tc.tile_pool(name="opool", bufs=3))
            pps = octx.enter_context(
                tc.tile_pool(name="proj_ps", bufs=2, space="PSUM"))
            stps = octx.enter_context(
                tc.tile_pool(name="st_ps", bufs=2, space="PSUM"))
            ctxps = octx.enter_context(
                tc.tile_pool(name="ctx_ps", bufs=2, space="PSUM"))
            rsps = octx.enter_context(
                tc.tile_pool(name="rs_ps", bufs=2, space="PSUM"))

            negmask = const.tile([128, 128], F32)
            onesc = const.tile([128, 1], F32R)
            onesr = const.tile([1, 128], F32R)
            ident = const.tile([128, 128], F32)
            nc.sync.dma_start(negmask[:], NM[:, :])
            nc.sync.dma_start(onesc[:], ONESC[:, :])
            nc.sync.dma_start(onesr[:], ONESR[:, :])
            nc.sync.dma_start(ident[:], IDENT[:, :])

            # Resident: weights, kT, vcat (keys/values reused across chunks)
            wq_t = [wpool.tile([128, NHL * HD], F32R, name=f"wq{d}",
                               tag=f"wq{d}") for d in range(NDT)]
            wk_t = [wpool.tile([128, HD], F32R, name=f"wk{d}",
                               tag=f"wk{d}") for d in range(NDT)]
            wv_t = [wpool.tile([128, HD], F32R, name=f"wv{d}",
                               tag=f"wv{d}") for d in range(NDT)]
            wo_t = [wpool.tile([128, D], F32R, name=f"wo{s}",
                               tag=f"wo{s}") for s in range(NHL)]
            kT = resid.tile([128, SEQ], F32R, name="kT", tag="kT")
            vcat = resid.tile([128, SEQ], F32R, name="vcat", tag="vcat")

            # Interleaved weight + first-chunk x DMAs: the first projection
            # matmul only needs (wk[0], xt0[0]) so compute starts ~2us in.
            xts = {}
            for d in range(NDT):
                nc.sync.dma_start(wk_t[d][:], Wk[d*128:(d+1)*128, :])
                nc.sync.dma_start(wv_t[d][:], Wv[d*128:(d+1)*128, :])
                nc.sync.dma_start(wq_t[d][:], Wq[d*128:(d+1)*128, :])
                xt = xpool.tile([128, QC], F32R, name=f"xt0_{d}", tag="xt")
                nc.sync.dma_start(xt[:], xT[d*128:(d+1)*128, 0:QC])
                xts[(0, d)] = xt

            qts = {}
            ctxs = {}
            for c in range(NQC):
                cs = slice(c * QC, (c + 1) * QC)
                kmax = 4 * (c + 1)

                # ---------------- projection, chunk c ----------------
                # block order: k, v first (attention on the diagonal needs
                # both immediately), then q heads.
                for blk in range(6):
                    ps = pps.tile([128, QC], F32, name=f"ps{c}_{blk}",
                                  tag="pp")
                    for d in range(NDT):
                        if blk == 0:
                            w = wk_t[d][:]
                        elif blk == 1:
                            w = wv_t[d][:]
                        else:
                            h = blk - 2
                            w = wq_t[d][:, h*HD:(h+1)*HD]
                        nc.tensor.matmul(ps[:], w, xts[(c, d)][:],
                                         start=(d == 0), stop=(d == NDT - 1))
                    if blk == 0:
                        nc.any.tensor_copy(kT[:, cs], ps[:])
                    elif blk == 1:
                        vt = vtpool.tile([128, QC], F32, name=f"vt{c}",
                                         tag="vt")
                        nc.any.tensor_copy(vt[:], ps[:])
                        for t in range(QC // 128):
                            trp = stps.tile([128, 128], F32,
                                            name=f"tr{c}_{t}", tag="st")
                            nc.tensor.transpose(trp[:], vt[:, t*128:(t+1)*128],
                                                ident[:])
                            col = c * QC + t * 128
                            nc.any.tensor_copy(vcat[:, col:col+128], trp[:])
                    else:
                        h = blk - 2
                        qt = qpool.tile([128, QC], F32R, name=f"qt{c}_{h}",
                                        tag="qt")
                        nc.any.tensor_copy(qt[:], ps[:])
                        qts[(c, h)] = qt

                # prefetch next chunk's x
                if c + 1 < NQC:
                    for d in range(NDT):
                        xt = xpool.tile([128, QC], F32R, name=f"xt{c+1}_{d}",
                                        tag="xt")
                        nc.sync.dma_start(xt[:],
                                          xT[d*128:(d+1)*128,
                                             (c+1)*QC:(c+2)*QC])
                        xts[(c + 1, d)] = xt
                if c == 0:
                    for s in range(NHL):
                        nc.sync.dma_start(wo_t[s][:], Wo[s*128:(s+1)*128, :])

                # ---------------- attention, chunk c ----------------
                recips = {}
                for h in range(NHL):
                    ctxp = ctxps.tile([128, QC], F32, name=f"ctxp{c}_{h}",
                                      tag="ctxp")
                    rsp = rsps.tile([1, QC], F32, name=f"rsp{c}_{h}",
                                    tag="rsp")
                    for ki in range(kmax):
                        j = ki - 4 * c
                        n0 = 0 if j < 0 else 128 * j
                        ns = slice(n0, QC)
                        stt = stps.tile([128, QC], F32,
                                        name=f"st{c}_{h}_{ki}", tag="st")
                        nc.tensor.matmul(stt[:, ns],
                                         kT[:, ki*128:(ki+1)*128],
                                         qts[(c, h)][:, n0:QC],
                                         start=True, stop=True)
                        if j >= 0:
                            nc.vector.tensor_tensor(
                                stt[:, n0:n0+128], stt[:, n0:n0+128],
                                negmask[:], ALU.add)
                        est = epool.tile([128, QC], F32R,
                                         name=f"est{c}_{h}_{ki}", tag="est")
                        nc.scalar.activation(est[:, ns], stt[:, ns],
                                             AF.Exp, scale=SCALE)
                        nc.tensor.matmul(ctxp[:, ns],
                                         vcat[:, ki*128:(ki+1)*128],
                                         est[:, ns],
                                         start=(ki == 0),
                                         stop=(ki == kmax - 1))
                        nc.tensor.matmul(rsp[:, ns], onesc[:], est[:, ns],
                                         start=(ki == 0),
                                         stop=(ki == kmax - 1))
                    recip = rpool.tile([1, QC], F32, name=f"rc{c}_{h}",
                                       tag="recip")
                    nc.vector.reciprocal_approx_fast(recip[:], rsp[:])
                    recips[h] = recip
                    ctx_t = cxpool.tile([128, QC], F32R, name=f"cx{c}_{h}",
                                        tag="ctx")
                    nc.any.tensor_copy(ctx_t[:], ctxp[:])
                    ctxs[(c, h)] = ctx_t

                # normalization (deferred, off the PE critical path):
                # GpSimd broadcasts each head's denominator row across
                # partitions, DVE multiplies in place.
                for h in range(NHL):
                    ctx_t = ctxs[(c, h)]
                    bc = bcpool.tile([128, QC], F32, name=f"bc{c}_{h}",
                                     tag="bc")
                    nc.gpsimd.partition_broadcast(bc[:], recips[h][:])
                    nc.vector.tensor_tensor(ctx_t[:], ctx_t[:], bc[:],
                                            ALU.mult)

                # ---------------- out-projection, chunk c ----------------
                for m in range(QC // 128):
                    row0 = c * QC + m * 128
                    msl = slice(m * 128, (m + 1) * 128)
                    for n in range(4):
                        pso = pps.tile([128, 512], F32,
                                       name=f"pso{c}_{m}_{n}", tag="pp")
                        for s in range(NHL):
                            nc.tensor.matmul(pso[:],
                                             ctxs[(c, s)][:, msl],
                                             wo_t[s][:, n*512:(n+1)*512],
                                             start=(s == 0),
                                             stop=(s == NHL - 1))
                        ot = opool.tile([128, 512], F32, name=f"ot{c}_{m}_{n}",
                                        tag="ot")
                        nc.any.tensor_copy(ot[:], pso[:])
                        nc.sync.dma_start(OUT[row0:row0+128,
                                              n*512:(n+1)*512], ot[:])

    nc.compile()
    nc.m = get_hw_module(nc.m)
    return nc


_NC = None


def _get_nc():
    global _NC
    if _NC is None:
        _NC = build_program()
    return _NC


def _consts():
    negmask = np.where(np.arange(128)[:, None] <= np.arange(128)[None, :],
                       0.0, NEG).astype(np.float32)
    return {
        "NM": negmask,
        "ONESC": np.ones((128, 1), np.float32),
        "ONESR": np.ones((1, 128), np.float32),
        "IDENT": np.eye(128, dtype=np.float32),
    }


def _make_in_maps(x, Wq, Wk, Wv, Wo):
    consts = _consts()
    b = x.shape[0]
    xTs = [np.ascontiguousarray(np.asarray(x[i]).T) for i in range(b)]
    in_maps = []
    for i in range(8):
        bi, g = i // 4, i % 4
        in_maps.append({
            "xT": xTs[bi],
            "Wq": np.ascontiguousarray(Wq[:, g*512:(g+1)*512]),
            "Wk": np.ascontiguousarray(Wk[:, g*128:(g+1)*128]),
            "Wv": np.ascontiguousarray(Wv[:, g*128:(g+1)*128]),
            "Wo": np.ascontiguousarray(Wo[g*512:(g+1)*512, :]),
            **consts,
        })
    return in_maps


def kernel(x, Wq, Wk, Wv, Wo, bo):
    x = np.asarray(x, np.float32)
    Wq = np.asarray(Wq, np.float32)
    Wk = np.asarray(Wk, np.float32)
    Wv = np.asarray(Wv, np.float32)
    Wo = np.asarray(Wo, np.float32)
    bo = np.asarray(bo, np.float32)
    b = x.shape[0]
    nc = _get_nc()
    in_maps = _make_in_maps(x, Wq, Wk, Wv, Wo)
    res = bass_utils.run_bass_kernel_spmd(nc, in_maps,
                                          core_ids=list(range(8)),
                                          trace=False)
    out = np.zeros((b, SEQ, D), np.float32)
    for i in range(8):
        bi = i // 4
        out[bi] += res.results[i]["out"]
    out += bo[None, None, :]
    return out


# revision 17
# speedup vs baseline: 5.3278x; 1.0114x over previous
"""GroupedQueryAttention Trainium2 Bass kernel (v2, chunk-pipelined).

Sharding: 8 cores = 2 (batch) x 4 (KV groups). Each core computes, for its
(b, g): q/k/v projections for the group's 4 query heads + 1 kv head, causal
attention, and the partial output projection ctx_g @ Wo[g-rows]. Host sums
the 4 group partials per batch and adds the bias.

v2 layout: one chunk-major loop (512 queries per chunk) that runs
projection -> attention -> output projection per chunk, so the Tile
scheduler can overlap the ACT-bound attention phase of chunk c with the
PE-bound projection of chunk c+1 / out-projection of chunk c-1.
Softmax normalization is off the critical path: ctx is evicted
unnormalized, denominators go through reciprocal_approx_fast (DVE) and a
deferred broadcast-matmul + multiply.
"""
import sys
sys.path.insert(0, '/opt/trn_rl_repo')

import numpy as np
import concourse.bass as bass
import concourse.bacc as bacc
import concourse.tile as tile
import concourse.mybir as mybir
from concourse import bass_utils
from concourse.bass_interp import get_hw_module
from contextlib import ExitStack, nullcontext

F32 = mybir.dt.float32
F32R = mybir.dt.float32r
AF = mybir.ActivationFunctionType
ALU = mybir.AluOpType

SEQ = 2048
D = 2048
HD = 128          # head dim
NHL = 4           # query heads per core (group size)
QC = 512          # query chunk
NQC = SEQ // QC   # 4
NDT = D // 128    # 16 contraction tiles
SCALE = 1.0 / float(np.sqrt(HD))
NEG = -1e30


def build_program(niter=1):
    nc = bacc.Bacc("TRN2", target_bir_lowering=False, debug=False,
                   enable_asserts=False, num_devices=8)
    xT = nc.dram_tensor("xT", [D, SEQ], F32R, kind="ExternalInput").ap()
    Wq = nc.dram_tensor("Wq", [D, NHL * HD], F32R, kind="ExternalInput").ap()
    Wk = nc.dram_tensor("Wk", [D, HD], F32R, kind="ExternalInput").ap()
    Wv = nc.dram_tensor("Wv", [D, HD], F32R, kind="ExternalInput").ap()
    Wo = nc.dram_tensor("Wo", [NHL * HD, D], F32R, kind="ExternalInput").ap()
    NM = nc.dram_tensor("NM", [128, 128], F32, kind="ExternalInput").ap()
    ONESC = nc.dram_tensor("ONESC", [128, 1], F32R, kind="ExternalInput").ap()
    ONESR = nc.dram_tensor("ONESR", [1, 128], F32R, kind="ExternalInput").ap()
    IDENT = nc.dram_tensor("IDENT", [128, 128], F32, kind="ExternalInput").ap()
    OUT = nc.dram_tensor("out", [SEQ, D], F32, kind="ExternalOutput").ap()

    with tile.TileContext(nc) as tc:
        with (tc.For_i(0, niter, 1) if niter > 1 else nullcontext()):
          with ExitStack() as octx:
            const = octx.enter_context(tc.tile_pool(name="const", bufs=1))
            wpool = octx.enter_context(tc.tile_pool(name="wpool", bufs=1))
            resid = octx.enter_context(tc.tile_pool(name="resid", bufs=1))
            xpool = octx.enter_context(tc.tile_pool(name="xpool", bufs=18))
            bcpool = octx.enter_context(tc.tile_pool(name="bcpool", bufs=2))
            qpool = octx.enter_context(tc.tile_pool(name="qpool", bufs=8))
            cxpool = octx.enter_context(tc.tile_pool(name="cxpool", bufs=8))
            epool = octx.enter_context(tc.tile_pool(name="epool", bufs=3))
            vtpool = octx.enter_context(tc.tile_pool(name="vtpool", bufs=2))
            rpool = octx.enter_context(tc.tile_pool(name="rpool", bufs=8))
            opool = octx.enter_context(tc.tile_pool(name="opool", bufs=3))
            pps = octx.enter_context(
                tc.tile_pool(name="proj_ps", bufs=2, space="PSUM"))
            stps = octx.enter_context(
                tc.tile_pool(name="st_ps", bufs=2, space="PSUM"))
            ctxps = octx.enter_context(
                tc.tile_pool(name="ctx_ps", bufs=2, space="PSUM"))
            rsps = octx.enter_context(
                tc.tile_pool(name="rs_ps", bufs=2, space="PSUM"))

            negmask = const.tile([128, 128], F32)
            onesc = const.tile([128, 1], F32R)
            onesr = const.tile([1, 128], F32R)
            ident = const.tile([128, 128], F32)
            nc.sync.dma_start(negmask[:], NM[:, :])
            nc.sync.dma_start(onesc[:], ONESC[:, :])
            nc.sync.dma_start(onesr[:], ONESR[:, :])
            nc.sync.dma_start(ident[:], IDENT[:, :])

            # Resident: weights, kT, vcat (keys/values reused across chunks)
            wq_t = [wpool.tile([128, NHL * HD], F32R, name=f"wq{d}",
                               tag=f"wq{d}") for d in range(NDT)]
            wk_t = [wpool.tile([128, HD], F32R, name=f"wk{d}",
                               tag=f"wk{d}") for d in range(NDT)]
            wv_t = [wpool.tile([128, HD], F32R, name=f"wv{d}",
                               tag=f"wv{d}") for d in range(NDT)]
            wo_t = [wpool.tile([128, D], F32R, name=f"wo{s}",
                               tag=f"wo{s}") for s in range(NHL)]
            kT = resid.tile([128, SEQ], F32R, name="kT", tag="kT")
            vcat = resid.tile([128, SEQ], F32R, name="vcat", tag="vcat")

            # Interleaved weight + first-chunk x DMAs: the first projection
            # matmul only needs (wk[0], xt0[0]) so compute starts ~2us in.
            xts = {}
            for d in range(NDT):
                nc.sync.dma_start(wk_t[d][:], Wk[d*128:(d+1)*128, :])
                nc.sync.dma_start(wv_t[d][:], Wv[d*128:(d+1)*128, :])
                nc.sync.dma_start(wq_t[d][:], Wq[d*128:(d+1)*128, :])
                xt = xpool.tile([128, QC], F32R, name=f"xt0_{d}", tag="xt")
                nc.sync.dma_start(xt[:], xT[d*128:(d+1)*128, 0:QC])
                xts[(0, d)] = xt

            qts = {}
            ctxs = {}
            for c in range(NQC):
                cs = slice(c * QC, (c + 1) * QC)
                kmax = 4 * (c + 1)

                # ---------------- projection, chunk c ----------------
                # block order: k, v first (attention on the diagonal needs
                # both immediately), then q heads.
                for blk in range(6):
                    ps = pps.tile([128, QC], F32, name=f"ps{c}_{blk}",
                                  tag="pp")
                    for d in range(NDT):
                        if blk == 0:
                            w = wk_t[d][:]
                        elif blk == 1:
                            w = wv_t[d][:]
                        else:
                            h = blk - 2
                            w = wq_t[d][:, h*HD:(h+1)*HD]
                        nc.tensor.matmul(ps[:], w, xts[(c, d)][:],
                                         start=(d == 0), stop=(d == NDT - 1))
                    if blk == 0:
                        nc.vector.tensor_copy(kT[:, cs], ps[:])
                    elif blk == 1:
                        vt = vtpool.tile([128, QC], F32, name=f"vt{c}",
                                         tag="vt")
                        nc.vector.tensor_copy(vt[:], ps[:])
                        for t in range(QC // 128):
                            trp = stps.tile([128, 128], F32,
                                            name=f"tr{c}_{t}", tag="st")
                            nc.tensor.transpose(trp[:], vt[:, t*128:(t+1)*128],
                                                ident[:])
                            col = c * QC + t * 128
                            nc.vector.tensor_copy(vcat[:, col:col+128], trp[:])
                    else:
                        h = blk - 2
                        qt = qpool.tile([128, QC], F32R, name=f"qt{c}_{h}",
                                        tag="qt")
                        nc.vector.tensor_copy(qt[:], ps[:])
                        qts[(c, h)] = qt

                # prefetch next chunk's x
                if c + 1 < NQC:
                    for d in range(NDT):
                        xt = xpool.tile([128, QC], F32R, name=f"xt{c+1}_{d}",
                                        tag="xt")
                        nc.sync.dma_start(xt[:],
                                          xT[d*128:(d+1)*128,
                                             (c+1)*QC:(c+2)*QC])
                        xts[(c + 1, d)] = xt
                if c == 0:
                    for s in range(NHL):
                        nc.sync.dma_start(wo_t[s][:], Wo[s*128:(s+1)*128, :])

                # ---------------- attention, chunk c ----------------
                recips = {}
                for h in range(NHL):
                    ctxp = ctxps.tile([128, QC], F32, name=f"ctxp{c}_{h}",
                                      tag="ctxp")
                    rsp = rsps.tile([1, QC], F32, name=f"rsp{c}_{h}",
                                    tag="rsp")
                    for ki in range(kmax):
                        j = ki - 4 * c
                        n0 = 0 if j < 0 else 128 * j
                        ns = slice(n0, QC)
                        stt = stps.tile([128, QC], F32,
                                        name=f"st{c}_{h}_{ki}", tag="st")
                        nc.tensor.matmul(stt[:, ns],
                                         kT[:, ki*128:(ki+1)*128],
                                         qts[(c, h)][:, n0:QC],
                                         start=True, stop=True)
                        if j >= 0:
                            nc.vector.tensor_tensor(
                                stt[:, n0:n0+128], stt[:, n0:n0+128],
                                negmask[:], ALU.add)
                        est = epool.tile([128, QC], F32R,
                                         name=f"est{c}_{h}_{ki}", tag="est")
                        nc.scalar.activation(est[:, ns], stt[:, ns],
                                             AF.Exp, scale=SCALE)
                        nc.tensor.matmul(ctxp[:, ns],
                                         vcat[:, ki*128:(ki+1)*128],
                                         est[:, ns],
                                         start=(ki == 0),
                                         stop=(ki == kmax - 1))
                        nc.tensor.matmul(rsp[:, ns], onesc[:], est[:, ns],
                                         start=(ki == 0),
                                         stop=(ki == kmax - 1))
                    recip = rpool.tile([1, QC], F32, name=f"rc{c}_{h}",
                                       tag="recip")
                    nc.vector.reciprocal_approx_fast(recip[:], rsp[:])
                    recips[h] = recip
                    ctx_t = cxpool.tile([128, QC], F32R, name=f"cx{c}_{h}",
                                        tag="ctx")
                    nc.vector.tensor_copy(ctx_t[:], ctxp[:])
                    ctxs[(c, h)] = ctx_t

                # normalization (deferred, off the PE critical path):
                # GpSimd broadcasts each head's denominator row across
                # partitions, DVE multiplies in place.
                for h in range(NHL):
                    ctx_t = ctxs[(c, h)]
                    bc = bcpool.tile([128, QC], F32, name=f"bc{c}_{h}",
                                     tag="bc")
                    nc.gpsimd.partition_broadcast(bc[:], recips[h][:])
                    nc.vector.tensor_tensor(ctx_t[:], ctx_t[:], bc[:],
                                            ALU.mult)

                # ---------------- out-projection, chunk c ----------------
                for m in range(QC // 128):
                    row0 = c * QC + m * 128
                    msl = slice(m * 128, (m + 1) * 128)
                    for n in range(4):
                        pso = pps.tile([128, 512], F32,
                                       name=f"pso{c}_{m}_{n}", tag="pp")
                        for s in range(NHL):
                            nc.tensor.matmul(pso[:],
                                             ctxs[(c, s)][:, msl],
                                             wo_t[s][:, n*512:(n+1)*512],
                                             start=(s == 0),
                                             stop=(s == NHL - 1))
                        ot = opool.tile([128, 512], F32, name=f"ot{c}_{m}_{n}",
                                        tag="ot")
                        nc.vector.tensor_copy(ot[:], pso[:])
                        nc.scalar.dma_start(OUT[row0:row0+128,
                                                n*512:(n+1)*512], ot[:])

    nc.compile()
    nc.m = get_hw_module(nc.m)
    return nc


_NC = None


def _get_nc():
    global _NC
    if _NC is None:
        _NC = build_program()
    return _NC


def _consts():
    negmask = np.where(np.arange(128)[:, None] <= np.arange(128)[None, :],
                       0.0, NEG).astype(np.float32)
    return {
        "NM": negmask,
        "ONESC": np.ones((128, 1), np.float32),
        "ONESR": np.ones((1, 128), np.float32),
        "IDENT": np.eye(128, dtype=np.float32),
    }


def _make_in_maps(x, Wq, Wk, Wv, Wo):
    consts = _consts()
    b = x.shape[0]
    xTs = [np.ascontiguousarray(np.asarray(x[i]).T) for i in range(b)]
    in_maps = []
    for i in range(8):
        bi, g = i // 4, i % 4
        in_maps.append({
            "xT": xTs[bi],
            "Wq": np.ascontiguousarray(Wq[:, g*512:(g+1)*512]),
            "Wk": np.ascontiguousarray(Wk[:, g*128:(g+1)*128]),
            "Wv": np.ascontiguousarray(Wv[:, g*128:(g+1)*128]),
            "Wo": np.ascontiguousarray(Wo[g*512:(g+1)*512, :]),
            **consts,
        })
    return in_maps


def kernel(x, Wq, Wk, Wv, Wo, bo):
    x = np.asarray(x, np.float32)
    Wq = np.asarray(Wq, np.float32)
    Wk = np.asarray(Wk, np.float32)
    Wv = np.asarray(Wv, np.float32)
    Wo = np.asarray(Wo, np.float32)
    bo = np.asarray(bo, np.float32)
    b = x.shape[0]
    nc = _get_nc()
    in_maps = _make_in_maps(x, Wq, Wk, Wv, Wo)
    res = bass_utils.run_bass_kernel_spmd(nc, in_maps,
                                          core_ids=list(range(8)),
                                          trace=False)
    out = np.zeros((b, SEQ, D), np.float32)
    for i in range(8):
        bi = i // 4
        out[bi] += res.results[i]["out"]
    out += bo[None, None, :]
    return out
